# revision 2
# baseline (speedup 1.0000x reference)
"""3x3 morphological dilation (== 3x3 stride-1 max-pool) on Trainium2.

Input:  img [16, 8, 512, 512] f32 in [0, 1).
Output: out[b,c,y,x] = max over the 3x3 window of img (border padded with -2,
        which never wins since img >= 0).

Strategy (8 NeuronCores, data parallel over H; fp16 on device):
  - Each core gets 64 output rows + 1 halo row each side: strip [128, 66, 520]
    fp16, cols = [-2 border, x0..x511, -2 border, 6 pad]. Edge rows are
    replicated at the global top/bottom (max-equivalent to -2 padding).
  - Vertical 3-max via the pair trick: Q[j] = max(L[2j], L[2j+1]) (one 2x-mode
    tensor_tensor MAX per tile), then each output row is max(pair row, single
    row).
  - The vertical merge AND the horizontal 3-tap max run fused in a single
    custom DVE op (DILATE3_ANT, registered below): out[i] = max(v[i-2..i]),
    v[i] = max(Src0[i], Src1[i]), using swap-flop temporal taps. A hand
    written 2x_1p uop program gives 2 results/cycle, so the whole
    horizontal+merge pass costs 0.5 cycle/output on the DVE:
      out rows 2j   = DILATE3(Q[j],      L[2j+2])
      out rows 2j+1 = DILATE3(L[2j+1],   Q[j+1])
    The op streams rows back-to-back; the first 2 outputs of each row carry
    stale window taps and land in the 2 discarded border columns.
  - Pipeline: warm-up DMAs absorb HWDGE first-use latency; tile 0's load is
    split so compute starts early; stores split across both DMA queues.
"""

import dataclasses

import numpy as np

import concourse.bass as bass
import concourse.bass_isa as bass_isa
import concourse.tile as tile
from concourse import bacc, mybir
from concourse import dve_ops as _dve_ops
from concourse.bass import MemorySpace, assert_partition_dims_match
from concourse.bass_utils import run_bass_kernel_spmd
from concourse.dve_spec import Spec, Src0, Src1, maxx
from concourse.dve_uop import (
    ENABLE,
    AluInp,
    AluOp,
    DelayInp,
    DveOpSpec,
    InpSel,
    OutPath,
    OutSel,
    Trigger,
    UopConfig,
    UopDpConfig,
)

N_CORES = 8
B, C, H, W = 16, 8, 512, 512
NIMG = B * C                     # 128 -> partition dim
ROWS_PER_CORE = H // N_CORES     # 64
STRIP_ROWS = ROWS_PER_CORE + 2   # 66 (1 halo row each side)
TILE_PLAN = (6, 10, 22, 24, 2)   # output rows per tile (sums to 64)
WIN = 520                        # strip row width (514 data + 6 pad, 16B mult)
XOFF = 1                         # col of x0 inside the strip row
OOFF = XOFF + 1                  # first valid output col (right-aligned win)
F16 = mybir.dt.float16

# ---------------------------------------------------------------------------
# DILATE3_ANT: custom DVE op, out[i] = max(v[i-2], v[i-1], v[i]),
# v[i] = max(Src0[i], Src1[i]) over the flattened free-dim stream.
# ---------------------------------------------------------------------------

_OP_NAME = "DILATE3_ANT"


def _uop_1x() -> UopConfig:
    u = UopConfig()
    u.enable_input(InpSel.SRC_0, 1)  # -> PREV_DELAY_0 at blk0
    u.enable_input(InpSel.SRC_1, 2)  # -> PREV_DELAY_1 at blk0
    u.datapath_config[0] = UopDpConfig().enable_alu(
        AluOp.MAX, AluInp.PREV_DELAY_0, AluInp.PREV_DELAY_1
    )
    # swap-flop delay tap: emits v[i-1], latches v[i]; v[i] also to lane 0
    u.datapath_config[1] = (
        UopDpConfig()
        .enable_alu(AluOp.BYPASS, AluInp.CURR_SWAP_OUT, AluInp.PREV_ALU_OUT)
        .enable_delay_from_src(DelayInp.PREV_ALU_OUT, 0)
    )
    u.datapath_config[1].swap_enable = ENABLE
    # second tap: emits v[i-2], latches v[i-1]; v[i-1] to lane 1
    u.datapath_config[2] = (
        UopDpConfig()
        .enable_alu(AluOp.BYPASS, AluInp.CURR_SWAP_OUT, AluInp.PREV_ALU_OUT)
        .enable_delay_from_src(DelayInp.PREV_ALU_OUT, 1)
        .pass_through_delay(0)
    )
    u.datapath_config[2].swap_enable = ENABLE
    u.datapath_config[3] = (
        UopDpConfig()
        .enable_alu(AluOp.MAX, AluInp.PREV_ALU_OUT, AluInp.PREV_DELAY_1)
        .pass_through_delay(0)
    )
    u.datapath_config[4] = UopDpConfig().enable_alu(
        AluOp.MAX, AluInp.PREV_ALU_OUT, AluInp.PREV_DELAY_0
    )
    for k in (5, 6, 7):
        u.datapath_config[k] = UopDpConfig().pass_through_alu()
    u.enable_output(OutSel.ALU_OUT, OutPath.WR0_LO)
    u.require_inp0 = ENABLE
    u.require_inp1 = ENABLE
    u.trigger = (Trigger.SRC_TENSOR_DONE, Trigger.NONE, Trigger.NONE)
    u.validate("v3")
    return u


def _uop_2x() -> UopConfig:
    """Packed pairs: per cycle v_lo/v_hi; m = max(v[2c-1], v[2c]);
    out_lo = max(m, v[2c-2]); out_hi = max(m, v[2c+1])."""
    u = UopConfig()
    u.enable_input(InpSel.SRC_0, 1)     # PREV_DELAY_0: s0 lo
    u.enable_input(InpSel.SRC_1, 2)     # PREV_DELAY_1: s1 lo
    u.enable_input(InpSel.SRC_0_HI, 3)  # PREV_DELAY_2: s0 hi
    u.enable_input(InpSel.SRC_1_HI, 4)  # PREV_DELAY_3: s1 hi
    u.datapath_config[0] = (
        UopDpConfig()
        .enable_alu(AluOp.MAX, AluInp.PREV_DELAY_0, AluInp.PREV_DELAY_1)
        .pass_through_delay(2, 3)
    )
    u.datapath_config[1] = (
        UopDpConfig()
        .enable_alu(AluOp.MAX, AluInp.PREV_DELAY_2, AluInp.PREV_DELAY_3)
        .enable_delay_from_src(DelayInp.PREV_ALU_OUT, 0)
    )
    u.datapath_config[2] = (
        UopDpConfig()
        .enable_alu(AluOp.BYPASS, AluInp.CURR_SWAP_OUT, AluInp.PREV_ALU_OUT)
        .enable_delay_from_src(DelayInp.PREV_ALU_OUT, 1)
        .pass_through_delay(0)
    )
    u.datapath_config[2].swap_enable = ENABLE
    u.datapath_config[3] = (
        UopDpConfig()
        .enable_alu(AluOp.BYPASS, AluInp.CURR_SWAP_OUT, AluInp.PREV_DELAY_0)
        .enable_delay_from_src(DelayInp.PREV_ALU_OUT, 2)
        .pass_through_delay(0, 1)
    )
    u.datapath_config[3].swap_enable = ENABLE
    u.datapath_config[4] = (
        UopDpConfig()
        .enable_alu(AluOp.MAX, AluInp.PREV_DELAY_2, AluInp.PREV_DELAY_0)
        .enable_delay_from_src(DelayInp.PREV_ALU_OUT, 3)
        .pass_through_delay(1)
    )
    u.datapath_config[5] = (
        UopDpConfig()
        .enable_alu(AluOp.MAX, AluInp.PREV_ALU_OUT, AluInp.PREV_DELAY_3)
        .enable_delay_from_src(DelayInp.PREV_ALU_OUT, 4)
        .pass_through_delay(1)
    )
    u.datapath_config[6] = (
        UopDpConfig()
        .enable_alu(AluOp.MAX, AluInp.PREV_DELAY_4, AluInp.PREV_DELAY_1)
        .enable_delay_from_src(DelayInp.PREV_ALU_OUT, 5)
    )
    u.datapath_config[7] = UopDpConfig().pass_through_alu().pass_through_delay(5)
    u.enable_output(OutSel.DELAY_5, OutPath.WR0_LO)
    u.enable_output(OutSel.ALU_OUT, OutPath.WR0_HI)
    u.require_inp0 = ENABLE
    u.require_inp1 = ENABLE
    u.trigger = (Trigger.SRC_TENSOR_DONE, Trigger.NONE, Trigger.NONE)
    u.validate("v3")
    return u


class _DilateOp(_dve_ops.DveOp):
    """Hand-written uop programs; bypasses Spec->uop lowering + sha pin."""

    def compile(self, ver):
        key = (self.name, ver)
        cached = _dve_ops._COMPILE_CACHE.get(key)
        if cached is not None:
            return cached
        assert ver == "v3", f"{_OP_NAME} only has a v3 (TRN2) program"
        spec = DveOpSpec(
            name=self.name,
            opcode=_dve_ops.get_dve_sub_opcode(self.name),
            uops=[_uop_1x()],
            uops_2x=[_uop_2x()],
            perf_max=1,
            rd1_en=True,
        )
        spec.validate(ver)
        _dve_ops._COMPILE_CACHE[key] = spec
        return spec


def _register_dilate_op() -> "_DilateOp":
    for op in _dve_ops.OPS:
        if op.name == _OP_NAME:
            return op
    op = _DilateOp(
        _OP_NAME,
        # Elementwise stand-in Spec: consulted only for Src1/C2 presence.
        Spec(body=maxx(Src0, Src1), reference=None),
        subdim=False,
        uops_sha={},
    )
    _dve_ops.OPS.append(op)
    _dve_ops._SUB_OPCODE_FOR_NAME[_OP_NAME] = (
        _dve_ops._CUSTOM_DVE_ROW_BASE + len(_dve_ops.OPS) - 1
    )
    assert _dve_ops._SUB_OPCODE_FOR_NAME[_OP_NAME] < 0x20
    _dve_ops.CUSTOM_DVE_SPECS[_OP_NAME] = op.spec
    return op


_DILATE3 = _register_dilate_op()


def _emit_dilate3(vec, out, in0, in1):
    """out[i] = max(v[i-2..i]), v = max(in0, in1), flattened free dims."""
    bs = vec.bass
    if _OP_NAME not in bs.m.ant_custom_dve_ops:
        bs.m.ant_custom_dve_ops = sorted({*bs.m.ant_custom_dve_ops, _OP_NAME})
    for ap in (out, in0, in1):
        assert ap.space in (MemorySpace.SBUF, MemorySpace.PSUM)
        assert len(ap.shape) <= 3
    assert_partition_dims_match(out, in0, in1, error_msg_prefix="dilate3 ")
    shape = bass_isa.CustomDveShape.STT
    isa_opcode = bs.isa.Opcode[
        f"NEURON_ISA_TPB_OPCODE_CUSTOM_DVE_ANT_{shape.slot()}"
    ].value
    zero = mybir.ImmediateValue(dtype=mybir.dt.float32, value=0.0)
    return vec.add_instruction(
        bass_isa.InstCustomDveAnt(
            name=bs.get_next_instruction_name(),
            op_name=_OP_NAME,
            rd1_en=True,
            subdim=0,
            imm2=0.0,
            shape=shape,
            row=_dve_ops.get_dve_sub_opcode(_OP_NAME),
            isa_opcode=isa_opcode,
            perf_max=1,
            ins=[
                vec.lower_ap(in0, for_isa=True, opt=True),
                vec.lower_ap(in1, for_isa=True, opt=True),
                zero,
                zero,
            ],
            outs=[vec.lower_ap(out, for_isa=True, opt=True)],
        )
    )


# ---------------------------------------------------------------------------
# Kernel
# ---------------------------------------------------------------------------

_compiled = {}


def _build_nc():
    nc = bacc.Bacc(
        "TRN2",
        target_bir_lowering=False,
        debug=False,
        num_devices=N_CORES,
    )
    img = nc.dram_tensor(
        "img", [NIMG, STRIP_ROWS, WIN], F16, kind="ExternalInput"
    ).ap()
    out = nc.dram_tensor(
        "out", [NIMG, ROWS_PER_CORE, W], F16, kind="ExternalOutput"
    ).ap()

    max_r = max(TILE_PLAN)
    with tile.TileContext(nc) as tc:
        with (
            tc.tile_pool(name="pin", bufs=3) as pin,
            tc.tile_pool(name="pwork", bufs=1) as pwork,
            tc.tile_pool(name="pout", bufs=3) as pout,
        ):
            q = pwork.tile([NIMG, max_r // 2 + 1, WIN], F16)
            warm = pwork.tile([NIMG, 1, 4], F16)

            # Warm both HWDGE queues with tiny loads (see baseline notes).
            nc.sync.dma_start(warm[:, 0, 0:2], img[:, 0, 0:2])
            nc.scalar.dma_start(warm[:, 0, 2:4], img[:, 0, 2:4])

            r0s = [sum(TILE_PLAN[:i]) for i in range(len(TILE_PLAN))]
            tins = [
                pin.tile([NIMG, max_r + 2, WIN], F16, tag="tin", name=f"tin{i}")
                for i in range(len(TILE_PLAN))
            ]

            def load_tile(ti):
                rows = TILE_PLAN[ti] + 2
                r0 = r0s[ti]
                if ti == 0:
                    nc.sync.dma_start(tins[0][:, 0:4, :], img[:, 0:4, :])
                    nc.sync.dma_start(tins[0][:, 4:8, :], img[:, 4:8, :])
                    return
                nc.sync.dma_start(
                    tins[ti][:, 0:rows, :], img[:, r0 : r0 + rows, :]
                )

            load_tile(0)
            load_tile(1)
            load_tile(2)

            for ti, R in enumerate(TILE_PLAN):
                nq = R // 2 + 1
                r0 = r0s[ti]
                tin = tins[ti]
                # vertical pairs Q[j] = max(L[2j], L[2j+1]), j = 0..R/2.
                # Tile 0 arrives in two 4-row halves; pair each half as it
                # lands so compute starts earlier.
                if ti == 0:
                    assert R == 6
                    nc.vector.tensor_max(
                        q[:, 0:2, :], tin[:, 0:4:2, :], tin[:, 1:4:2, :]
                    )
                    nc.vector.tensor_max(
                        q[:, 2:4, :], tin[:, 4:8:2, :], tin[:, 5:8:2, :]
                    )
                else:
                    nc.vector.tensor_max(
                        q[:, 0:nq, :],
                        tin[:, 0 : R + 2 : 2, :],
                        tin[:, 1 : R + 2 : 2, :],
                    )

                # prefetch a later tile's load before this tile's stores
                if ti + 3 < len(TILE_PLAN):
                    load_tile(ti + 3)

                # fused vertical-merge + horizontal 3-max; split the big
                # second-to-last tile so its stores drain during compute
                split = ti == len(TILE_PLAN) - 2
                chunks = [(0, R - 10), (R - 10, R)] if split else [(0, R)]
                for ra, rb in chunks:
                    n = rb - ra
                    o = pout.tile([NIMG, 24, WIN], F16, tag="o")
                    # even rows: out[2j] = dilate3(Q[j], L[2j+2])
                    _emit_dilate3(
                        nc.vector,
                        o[:, 0:n:2, :],
                        q[:, ra // 2 : rb // 2, :],
                        tin[:, ra + 2 : rb + 2 : 2, :],
                    )
                    # odd rows: out[2j+1] = dilate3(L[2j+1], Q[j+1])
                    _emit_dilate3(
                        nc.vector,
                        o[:, 1:n:2, :],
                        tin[:, ra + 1 : rb + 1 : 2, :],
                        q[:, ra // 2 + 1 : rb // 2 + 1, :],
                    )
                    # store the 512 valid columns, split across both queues
                    half = n // 2
                    nc.sync.dma_start(
                        out[:, r0 + ra : r0 + ra + half, :],
                        o[:, 0:half, OOFF : OOFF + W],
                    )
                    nc.scalar.dma_start(
                        out[:, r0 + ra + half : r0 + rb, :],
                        o[:, half:n, OOFF : OOFF + W],
                    )

    nc.compile()
    return nc


def _get_nc():
    if "nc" not in _compiled:
        _compiled["nc"] = _build_nc()
    return _compiled["nc"]


def _prep(img: np.ndarray) -> list[dict]:
    """img f32 [B,C,H,W] -> 8 per-core strips [128, 66, 520] fp16:
    cols [-2, x0..x511, -2, 6x -2 pad], 1-row halo (edge-replicated at the
    global top/bottom, max-equivalent to -2 pad)."""
    flat = img.reshape(NIMG, H, W).astype(np.float16)
    P = np.full((NIMG, H, WIN), -2.0, dtype=np.float16)
    P[:, :, XOFF : XOFF + W] = flat
    shards = []
    for c in range(N_CORES):
        lo = c * ROWS_PER_CORE - 1
        hi = c * ROWS_PER_CORE + ROWS_PER_CORE + 1
        if lo < 0:
            strip = np.concatenate([P[:, :1], P[:, 0:hi]], axis=1)
        elif hi > H:
            strip = np.concatenate([P[:, lo:], P[:, H - 1 :]], axis=1)
        else:
            strip = P[:, lo:hi]
        shards.append(np.ascontiguousarray(strip))
    return [{"img": s} for s in shards]


def _post(parts: list[np.ndarray]) -> np.ndarray:
    """8 strips [128, 64, 512] fp16 -> [B,C,H,W] f32."""
    res = np.concatenate(parts, axis=1)
    return res.astype(np.float32).reshape(B, C, H, W)


def kernel(img: np.ndarray, **_unused) -> np.ndarray:
    img = np.asarray(img, dtype=np.float32)
    assert img.shape == (B, C, H, W), img.shape

    nc = _get_nc()
    in_maps = _prep(img)
    res = run_bass_kernel_spmd(nc, in_maps, core_ids=list(range(N_CORES)))
    parts = [res.results[k]["out"] for k in range(N_CORES)]
    return _post(parts)


# revision 5
# speedup vs baseline: 1.0585x; 1.0585x over previous
"""3x3 morphological dilation (== 3x3 stride-1 max-pool) on Trainium2.

Input:  img [16, 8, 512, 512] f32 in [0, 1).
Output: out[b,c,y,x] = max over the 3x3 window of img (border padded with -2,
        which never wins since img >= 0).

Strategy (8 NeuronCores, data parallel over H; fp16 on device):
  - Each core gets 64 output rows + 1 halo row each side: strip [128, 66, 514]
    fp16, cols = [-2 border, x0..x511, -2 border]. Edge rows are replicated at
    the global top/bottom (max-equivalent to -2 padding).
  - Vertical pairs A[y] = max(L[y], L[y+1]) for every y (one dense 2x-mode
    tensor_tensor MAX per tile), so the vertical 3-max of any output row is
    max(A[y], L[y+2]).
  - That merge AND the horizontal 3-tap max run fused in a single custom DVE
    op (DILATE3_ANT, registered below): out[i] = max(v[i-2..i]) with
    v[i] = max(Src0[i], Src1[i]), via swap-flop temporal taps. A hand-written
    2x_1p uop program gives 2 results/cycle.
  - The fused op writes a PACKED 512-wide flat output tile with DESCENDING
    row order: each stream row emits 514 values of which the leading 2 are
    stale-tap junk; with dst row stride -512 the junk lands on the
    not-yet-written previous row and is overwritten, so the tile ends up
    densely packed and the store coalesces into one big descriptor per
    partition (the 520-wide-tile variant of this kernel paid ~20us in 1KB
    store descriptors).
  - Pipeline: warm-up DMAs absorb HWDGE first-use latency; tile 0's load is
    split so compute starts early; stores split across both DMA queues.
"""

import dataclasses

import numpy as np

import concourse.bass as bass
import concourse.bass_isa as bass_isa
import concourse.tile as tile
from concourse import bacc, mybir
from concourse import dve_ops as _dve_ops
from concourse.bass import MemorySpace, assert_partition_dims_match
from concourse.bass_utils import run_bass_kernel_spmd
from concourse.dve_spec import Spec, Src0, Src1, maxx
from concourse.dve_uop import (
    ENABLE,
    AluInp,
    AluOp,
    DelayInp,
    DveOpSpec,
    InpSel,
    OutPath,
    OutSel,
    Trigger,
    UopConfig,
    UopDpConfig,
)

N_CORES = 8
B, C, H, W = 16, 8, 512, 512
NIMG = B * C                     # 128 -> partition dim
ROWS_PER_CORE = H // N_CORES     # 64
STRIP_ROWS = ROWS_PER_CORE + 2   # 66 (1 halo row each side)
TILE_PLAN = (6, 10, 22, 24, 2)   # output rows per tile (sums to 64)
S = 514                          # strip row width: [-2, x0..x511, -2]
F16 = mybir.dt.float16

# ---------------------------------------------------------------------------
# DILATE3_ANT: custom DVE op, out[i] = max(v[i-2], v[i-1], v[i]),
# v[i] = max(Src0[i], Src1[i]) over the flattened free-dim stream.
# ---------------------------------------------------------------------------

_OP_NAME = "DILATE3_ANT"


def _uop_1x() -> UopConfig:
    u = UopConfig()
    u.enable_input(InpSel.SRC_0, 1)  # -> PREV_DELAY_0 at blk0
    u.enable_input(InpSel.SRC_1, 2)  # -> PREV_DELAY_1 at blk0
    u.datapath_config[0] = UopDpConfig().enable_alu(
        AluOp.MAX, AluInp.PREV_DELAY_0, AluInp.PREV_DELAY_1
    )
    # swap-flop delay tap: emits v[i-1], latches v[i]; v[i] also to lane 0
    u.datapath_config[1] = (
        UopDpConfig()
        .enable_alu(AluOp.BYPASS, AluInp.CURR_SWAP_OUT, AluInp.PREV_ALU_OUT)
        .enable_delay_from_src(DelayInp.PREV_ALU_OUT, 0)
    )
    u.datapath_config[1].swap_enable = ENABLE
    # second tap: emits v[i-2], latches v[i-1]; v[i-1] to lane 1
    u.datapath_config[2] = (
        UopDpConfig()
        .enable_alu(AluOp.BYPASS, AluInp.CURR_SWAP_OUT, AluInp.PREV_ALU_OUT)
        .enable_delay_from_src(DelayInp.PREV_ALU_OUT, 1)
        .pass_through_delay(0)
    )
    u.datapath_config[2].swap_enable = ENABLE
    u.datapath_config[3] = (
        UopDpConfig()
        .enable_alu(AluOp.MAX, AluInp.PREV_ALU_OUT, AluInp.PREV_DELAY_1)
        .pass_through_delay(0)
    )
    u.datapath_config[4] = UopDpConfig().enable_alu(
        AluOp.MAX, AluInp.PREV_ALU_OUT, AluInp.PREV_DELAY_0
    )
    for k in (5, 6, 7):
        u.datapath_config[k] = UopDpConfig().pass_through_alu()
    u.enable_output(OutSel.ALU_OUT, OutPath.WR0_LO)
    u.require_inp0 = ENABLE
    u.require_inp1 = ENABLE
    u.trigger = (Trigger.SRC_TENSOR_DONE, Trigger.NONE, Trigger.NONE)
    u.validate("v3")
    return u


def _uop_2x() -> UopConfig:
    """Packed pairs: per cycle v_lo/v_hi; m = max(v[2c-1], v[2c]);
    out_lo = max(m, v[2c-2]); out_hi = max(m, v[2c+1])."""
    u = UopConfig()
    u.enable_input(InpSel.SRC_0, 1)     # PREV_DELAY_0: s0 lo
    u.enable_input(InpSel.SRC_1, 2)     # PREV_DELAY_1: s1 lo
    u.enable_input(InpSel.SRC_0_HI, 3)  # PREV_DELAY_2: s0 hi
    u.enable_input(InpSel.SRC_1_HI, 4)  # PREV_DELAY_3: s1 hi
    u.datapath_config[0] = (
        UopDpConfig()
        .enable_alu(AluOp.MAX, AluInp.PREV_DELAY_0, AluInp.PREV_DELAY_1)
        .pass_through_delay(2, 3)
    )
    u.datapath_config[1] = (
        UopDpConfig()
        .enable_alu(AluOp.MAX, AluInp.PREV_DELAY_2, AluInp.PREV_DELAY_3)
        .enable_delay_from_src(DelayInp.PREV_ALU_OUT, 0)
    )
    u.datapath_config[2] = (
        UopDpConfig()
        .enable_alu(AluOp.BYPASS, AluInp.CURR_SWAP_OUT, AluInp.PREV_ALU_OUT)
        .enable_delay_from_src(DelayInp.PREV_ALU_OUT, 1)
        .pass_through_delay(0)
    )
    u.datapath_config[2].swap_enable = ENABLE
    u.datapath_config[3] = (
        UopDpConfig()
        .enable_alu(AluOp.BYPASS, AluInp.CURR_SWAP_OUT, AluInp.PREV_DELAY_0)
        .enable_delay_from_src(DelayInp.PREV_ALU_OUT, 2)
        .pass_through_delay(0, 1)
    )
    u.datapath_config[3].swap_enable = ENABLE
    u.datapath_config[4] = (
        UopDpConfig()
        .enable_alu(AluOp.MAX, AluInp.PREV_DELAY_2, AluInp.PREV_DELAY_0)
        .enable_delay_from_src(DelayInp.PREV_ALU_OUT, 3)
        .pass_through_delay(1)
    )
    u.datapath_config[5] = (
        UopDpConfig()
        .enable_alu(AluOp.MAX, AluInp.PREV_ALU_OUT, AluInp.PREV_DELAY_3)
        .enable_delay_from_src(DelayInp.PREV_ALU_OUT, 4)
        .pass_through_delay(1)
    )
    u.datapath_config[6] = (
        UopDpConfig()
        .enable_alu(AluOp.MAX, AluInp.PREV_DELAY_4, AluInp.PREV_DELAY_1)
        .enable_delay_from_src(DelayInp.PREV_ALU_OUT, 5)
    )
    u.datapath_config[7] = UopDpConfig().pass_through_alu().pass_through_delay(5)
    u.enable_output(OutSel.DELAY_5, OutPath.WR0_LO)
    u.enable_output(OutSel.ALU_OUT, OutPath.WR0_HI)
    u.require_inp0 = ENABLE
    u.require_inp1 = ENABLE
    u.trigger = (Trigger.SRC_TENSOR_DONE, Trigger.NONE, Trigger.NONE)
    u.validate("v3")
    return u


class _DilateOp(_dve_ops.DveOp):
    """Hand-written uop programs; bypasses Spec->uop lowering + sha pin."""

    def compile(self, ver):
        key = (self.name, ver)
        cached = _dve_ops._COMPILE_CACHE.get(key)
        if cached is not None:
            return cached
        assert ver == "v3", f"{_OP_NAME} only has a v3 (TRN2) program"
        spec = DveOpSpec(
            name=self.name,
            opcode=_dve_ops.get_dve_sub_opcode(self.name),
            uops=[_uop_1x()],
            uops_2x=[_uop_2x()],
            perf_max=1,
            rd1_en=True,
        )
        spec.validate(ver)
        _dve_ops._COMPILE_CACHE[key] = spec
        return spec


def _register_dilate_op() -> "_DilateOp":
    for op in _dve_ops.OPS:
        if op.name == _OP_NAME:
            return op
    op = _DilateOp(
        _OP_NAME,
        # Elementwise stand-in Spec: consulted only for Src1/C2 presence.
        Spec(body=maxx(Src0, Src1), reference=None),
        subdim=False,
        uops_sha={},
    )
    _dve_ops.OPS.append(op)
    _dve_ops._SUB_OPCODE_FOR_NAME[_OP_NAME] = (
        _dve_ops._CUSTOM_DVE_ROW_BASE + len(_dve_ops.OPS) - 1
    )
    assert _dve_ops._SUB_OPCODE_FOR_NAME[_OP_NAME] < 0x20
    _dve_ops.CUSTOM_DVE_SPECS[_OP_NAME] = op.spec
    return op


_DILATE3 = _register_dilate_op()


def _emit_dilate3(vec, out, in0, in1):
    """out[i] = max(v[i-2..i]), v = max(in0, in1), flattened free dims."""
    bs = vec.bass
    if _OP_NAME not in bs.m.ant_custom_dve_ops:
        bs.m.ant_custom_dve_ops = sorted({*bs.m.ant_custom_dve_ops, _OP_NAME})
    for ap in (out, in0, in1):
        assert ap.space in (MemorySpace.SBUF, MemorySpace.PSUM)
        assert len(ap.shape) <= 3
    assert_partition_dims_match(out, in0, in1, error_msg_prefix="dilate3 ")
    shape = bass_isa.CustomDveShape.STT
    isa_opcode = bs.isa.Opcode[
        f"NEURON_ISA_TPB_OPCODE_CUSTOM_DVE_ANT_{shape.slot()}"
    ].value
    zero = mybir.ImmediateValue(dtype=mybir.dt.float32, value=0.0)
    return vec.add_instruction(
        bass_isa.InstCustomDveAnt(
            name=bs.get_next_instruction_name(),
            op_name=_OP_NAME,
            rd1_en=True,
            subdim=0,
            imm2=0.0,
            shape=shape,
            row=_dve_ops.get_dve_sub_opcode(_OP_NAME),
            isa_opcode=isa_opcode,
            perf_max=1,
            ins=[
                vec.lower_ap(in0, for_isa=True, opt=True),
                vec.lower_ap(in1, for_isa=True, opt=True),
                zero,
                zero,
            ],
            outs=[vec.lower_ap(out, for_isa=True, opt=True)],
        )
    )


def _desc_rows(last_row_view, n, row_stride):
    """Rank-3 AP iterating n rows in DESCENDING memory order. `last_row_view`
    is the [NIMG, width] slice of the HIGHEST-address row; inserts a
    [-row_stride, n] dim (strides in elements) after the partition dim."""
    ap = [list(x) for x in last_row_view.ap]
    ap.insert(1, [-row_stride, n])
    return dataclasses.replace(last_row_view, ap=ap)


# ---------------------------------------------------------------------------
# Kernel
# ---------------------------------------------------------------------------

_compiled = {}


def _build_nc():
    nc = bacc.Bacc(
        "TRN2",
        target_bir_lowering=False,
        debug=False,
        num_devices=N_CORES,
    )
    img = nc.dram_tensor(
        "img", [NIMG, STRIP_ROWS, S], F16, kind="ExternalInput"
    ).ap()
    out = nc.dram_tensor(
        "out", [NIMG, ROWS_PER_CORE, W], F16, kind="ExternalOutput"
    ).ap()

    max_r = max(TILE_PLAN)
    with tile.TileContext(nc) as tc:
        with (
            tc.tile_pool(name="pin", bufs=3) as pin,
            tc.tile_pool(name="pwork", bufs=1) as pwork,
            tc.tile_pool(name="pout", bufs=3) as pout,
        ):
            # every-row pairs A[y] = max(L[y], L[y+1]), y = 0..R
            a = pwork.tile([NIMG, max_r + 1, S], F16)
            warm = pwork.tile([NIMG, 1, 4], F16)

            # Warm both HWDGE queues with tiny loads.
            nc.sync.dma_start(warm[:, 0, 0:2], img[:, 0, 0:2])
            nc.scalar.dma_start(warm[:, 0, 2:4], img[:, 0, 2:4])

            r0s = [sum(TILE_PLAN[:i]) for i in range(len(TILE_PLAN))]
            tins = [
                pin.tile([NIMG, max_r + 2, S], F16, tag="tin", name=f"tin{i}")
                for i in range(len(TILE_PLAN))
            ]

            def load_tile(ti):
                rows = TILE_PLAN[ti] + 2
                r0 = r0s[ti]
                if ti == 0:
                    nc.sync.dma_start(tins[0][:, 0:4, :], img[:, 0:4, :])
                    nc.sync.dma_start(tins[0][:, 4:8, :], img[:, 4:8, :])
                    return
                nc.sync.dma_start(
                    tins[ti][:, 0:rows, :], img[:, r0 : r0 + rows, :]
                )

            load_tile(0)
            load_tile(1)
            load_tile(2)

            for ti, R in enumerate(TILE_PLAN):
                r0 = r0s[ti]
                tin = tins[ti]
                # pairs A[y] = max(L[y], L[y+1]), y = 0..R (dense, one op).
                # Tile 0 arrives in two 4-row halves; pair each half as it
                # lands so compute starts earlier.
                if ti == 0:
                    assert R == 6
                    nc.vector.tensor_max(
                        a[:, 0:3, :], tin[:, 0:3, :], tin[:, 1:4, :]
                    )
                    nc.vector.tensor_max(
                        a[:, 3:7, :], tin[:, 3:7, :], tin[:, 4:8, :]
                    )
                else:
                    nc.vector.tensor_max(
                        a[:, 0 : R + 1, :],
                        tin[:, 0 : R + 1, :],
                        tin[:, 1 : R + 2, :],
                    )

                # prefetch a later tile's load before this tile's stores
                if ti + 3 < len(TILE_PLAN):
                    load_tile(ti + 3)

                # fused vertical-merge + horizontal 3-max, all R rows in one
                # op, rows descending into a packed flat out tile.
                # out[y] = dilate3(A[y], L[y+2]); stream row y emits 514
                # values at flat offsets y*512 + 0..513; the leading 2 are
                # junk overwriting row y-1's tail cols, which row y-1
                # (processed later) then rewrites correctly.
                split = ti == len(TILE_PLAN) - 2
                chunks = [(0, R - 10), (R - 10, R)] if split else [(0, R)]
                for ra, rb in chunks:
                    n = rb - ra
                    o = pout.tile([NIMG, 2 + 24 * W], F16, tag="o")
                    _emit_dilate3(
                        nc.vector,
                        _desc_rows(o[:, (n - 1) * W : (n - 1) * W + S], n, W),
                        _desc_rows(a[:, ra + n - 1, :], n, S),
                        _desc_rows(tin[:, ra + n + 1, :], n, S),
                    )
                    # coalesced store of the packed [n, 512] block, split
                    # across both queues
                    half = n // 2
                    nc.sync.dma_start(
                        out[:, r0 + ra : r0 + ra + half, :],
                        o[:, 2 : 2 + half * W],
                    )
                    nc.scalar.dma_start(
                        out[:, r0 + ra + half : r0 + rb, :],
                        o[:, 2 + half * W : 2 + n * W],
                    )

    nc.compile()
    return nc


def _get_nc():
    if "nc" not in _compiled:
        _compiled["nc"] = _build_nc()
    return _compiled["nc"]


def _prep(img: np.ndarray) -> list[dict]:
    """img f32 [B,C,H,W] -> 8 per-core strips [128, 66, 514] fp16:
    cols [-2, x0..x511, -2], 1-row halo (edge-replicated at the global
    top/bottom, max-equivalent to -2 pad)."""
    flat = img.reshape(NIMG, H, W).astype(np.float16)
    P = np.full((NIMG, H, S), -2.0, dtype=np.float16)
    P[:, :, 1 : 1 + W] = flat
    shards = []
    for c in range(N_CORES):
        lo = c * ROWS_PER_CORE - 1
        hi = c * ROWS_PER_CORE + ROWS_PER_CORE + 1
        if lo < 0:
            strip = np.concatenate([P[:, :1], P[:, 0:hi]], axis=1)
        elif hi > H:
            strip = np.concatenate([P[:, lo:], P[:, H - 1 :]], axis=1)
        else:
            strip = P[:, lo:hi]
        shards.append(np.ascontiguousarray(strip))
    return [{"img": s} for s in shards]


def _post(parts: list[np.ndarray]) -> np.ndarray:
    """8 strips [128, 64, 512] fp16 -> [B,C,H,W] f32."""
    res = np.concatenate(parts, axis=1)
    return res.astype(np.float32).reshape(B, C, H, W)


def kernel(img: np.ndarray, **_unused) -> np.ndarray:
    img = np.asarray(img, dtype=np.float32)
    assert img.shape == (B, C, H, W), img.shape

    nc = _get_nc()
    in_maps = _prep(img)
    res = run_bass_kernel_spmd(nc, in_maps, core_ids=list(range(N_CORES)))
    parts = [res.results[k]["out"] for k in range(N_CORES)]
    return _post(parts)
